# revision 27
# baseline (speedup 1.0000x reference)
"""Trainium2 Bass kernel for the vq_codebook classifier problem.

Computes, for X [4096, 512] f32 and grp [1, 512, 100] f32:
    l1   = sum_d |X[n,d] - grp[0,d,c]|            -> [N, C]
    norm = softmax(-l1, axis=1)
    cs   = (X @ g) / max(|X| * |g|, eps)           (cosine similarity)
    out  = max_c(cs) * softmax(cs, axis=1) * norm

Sharding: data-parallel over N across 8 NeuronCores (512 rows each),
grp replicated.

Math notes used by the kernel:
  |v| = 2*relu(v) - v, so
  l1[n,c] = 2*sum_d relu(x-g) - sum_d x + sum_d g
The "sum_d x" term is constant over classes and cancels inside
softmax(-l1), so it is dropped. sum_d g (G1) is injected into the same
PSUM accumulator via a rank-1 matmul (ones-row x 0.5*G1).

relu(x - g) tiles are produced in a d-on-partition layout by the vector
engine (tensor_scalar sub+max, f16 2x mode, ~69% of tiles) and the
scalar engine (activation Relu with per-partition bias -g, ~31%). The
partition (d) reduction runs on TensorE with the relu tile as the
*stationary* operand (f16 fast-weight-load) and a ones-column as the
moving operand, so each class lands in one PSUM *column* of an [n, c]
accumulator (PE outputs may only start at partition 0/32/64, but
free-dim offsets are unrestricted; this also leaves results
pre-transposed for the per-row epilogue).

Scheduling notes (the engines are all near-saturated, so ordering is
what matters):
  - reps>1 (timing harness) runs the whole body in a tc.For_i hardware
    loop: NEFF size, and thus host-side per-call cost, is independent
    of reps, so the wall-clock differential isolates device time.
  - ~32 f16 warm-up matmuls at the head of each rep trip the PE HAM
    clock gate (1.2 -> 2.4 GHz) while the input DMAs land.
  - The X^T transposes write four row-chunks into one PSUM bank so a
    single wide copy casts each [128, 512] tile to f16.
  - The cosine/ghat chain (G^T, column norms, ghat, dot matmuls) is
    dribbled into the class loop a few classes apart; the in-order PE
    queue reaches each piece long after its inputs are ready, so the
    1600-matmul reduce stream never stalls on it.
  - The G1 inject row is reduced straight from the g tiles with four
    ones-stationary matmuls, so it only depends on the G DMA.
  - Deep u-tile pool (48) lets producers run ~12 us ahead of the PE.
"""

import numpy as np

P = 128
R = 512          # rows per core (4096 / 8 cores)
D = 512
C = 100
NT = D // P      # 4 d-tiles
RT = R // P      # 4 row-tiles
N_CORES = 8

_CACHE = {}
import os as _os
WARMUP_MMS = int(_os.environ.get("WARMUP_MMS", "32"))


def _split_excess_waits(nc, limit=1):
    """walrus in this container rejects instructions carrying more than
    one sync wait ("Too many sync wait commands"). Hoist excess waits
    onto same-engine NoOps inserted immediately before the instruction."""
    import concourse.mybir as mb
    import bass_rust

    n_id = [0]

    def mknop(engine, waits):
        n_id[0] += 1
        return bass_rust.InstNoOp(
            name=f"waitsplit-{n_id[0]}", engine=engine, ins=[], outs=[],
            sync_info=mb.SyncInfo(on_wait=list(waits), on_update=[]),
        )

    for fn in nc.m.functions:
        for bb in fn.blocks:
            insts = bb.instructions
            out = []
            for inst in insts:
                si = inst.sync_info
                if si is not None and si.on_wait and len(si.on_wait) > limit:
                    waits = list(si.on_wait)
                    extra, keep = waits[:-limit], waits[-limit:]
                    for w in extra:
                        out.append(mknop(inst.engine, [w]))
                    inst.sync_info = mb.SyncInfo(
                        on_wait=keep, on_update=list(si.on_update)
                    )
                out.append(inst)
            insts[:] = out


def _build_nc(reps: int = 1):
    import concourse.bass as bass
    import concourse.mybir as mybir
    import concourse.tile as tile
    from concourse.masks import make_identity
    from contextlib import ExitStack

    f32 = mybir.dt.float32
    f16 = mybir.dt.float16
    Alu = mybir.AluOpType
    Act = mybir.ActivationFunctionType
    Ax = mybir.AxisListType

    nc = bass.Bass(target_bir_lowering=False)
    Xd = nc.declare_dram_parameter("X", [R, D], f32, isOutput=False)
    Gd = nc.declare_dram_parameter("G", [D, C], f32, isOutput=False)
    Yd = nc.declare_dram_parameter("Y", [R, C], f32, isOutput=True)

    with ExitStack() as ctx:
        tc = ctx.enter_context(tile.TileContext(nc))
        consts = ctx.enter_context(tc.tile_pool(name="consts", bufs=1))
        xr_pool = ctx.enter_context(tc.tile_pool(name="xr", bufs=RT))
        xt_pool = ctx.enter_context(tc.tile_pool(name="xt", bufs=NT))
        g_pool = ctx.enter_context(tc.tile_pool(name="g", bufs=NT))
        gnb_pool = ctx.enter_context(tc.tile_pool(name="gnb", bufs=NT))
        gh_pool = ctx.enter_context(tc.tile_pool(name="gh", bufs=NT))
        small = ctx.enter_context(tc.tile_pool(name="small", bufs=24))
        scratch = ctx.enter_context(tc.tile_pool(name="scratch", bufs=2))
        u_pool = ctx.enter_context(tc.tile_pool(name="u", bufs=48))
        out_pool = ctx.enter_context(tc.tile_pool(name="out", bufs=RT))

        from contextlib import nullcontext

        def _rep_scope():
            # reps>1 exists only for the timing harness: run the whole
            # computation `reps` times on-device via a hardware loop so the
            # NEFF (and thus host-side lowering/transfer cost) stays
            # constant in `reps` and the differential isolates HW time.
            return tc.For_i(0, reps, 1) if reps > 1 else nullcontext()

        # ---- constants (loop-invariant; built once) ----
        ident = consts.tile([P, P], f32)
        make_identity(nc, ident[:])
        ones_col = consts.tile([P, 1], f16)
        nc.vector.memset(ones_col[:], 1.0)
        ones_row = consts.tile([1, P], f32)
        nc.vector.memset(ones_row[:], 1.0)
        ones_colf = consts.tile([P, 1], f32)
        nc.vector.memset(ones_colf[:], 1.0)
        ident16 = consts.tile([P, P], f16)
        nc.vector.tensor_copy(ident16[:], ident[:])

        with _rep_scope():
            # ---- load inputs (spread across the two hwdge rings) ----
            xr = []
            for k in range(RT):
                t = xr_pool.tile([P, D], f32, tag="xr", name=f"xr{k}")
                eng = nc.sync if k % 2 == 0 else nc.scalar
                eng.dma_start(t[:], Xd[k * P:(k + 1) * P, :])
                xr.append(t)
            g = []
            for t_ in range(NT):
                gt = g_pool.tile([P, C], f32, tag="g", name=f"g{t_}")
                nc.sync.dma_start(gt[:], Gd[t_ * P:(t_ + 1) * P, :])
                g.append(gt)

            # warm the PE clock gate (HAM) while the DMAs land: ~40 dummy
            # matmuls on a constant tile keep the PE busy through the first
            # activity window so the real matmuls run at 2.4 GHz
            with tc.tile_pool(name="warm_ps", bufs=1, space="PSUM") as warm_ps:
                wps = warm_ps.tile([P, P], f32, tag="warm")
                for _w in range(WARMUP_MMS):
                    nc.tensor.matmul(
                        wps[:], lhsT=ident16[:], rhs=ident16[:],
                        start=True, stop=True,
                    )

            xt = [xt_pool.tile([P, R], f16, tag="xt", name=f"xt{i}") for i in range(NT)]
            gT = consts.tile([C, D], f32)
            gh_all = gh_pool.tile([P, NT * C], f16, tag="gh", name="gh_all")
            g1row = consts.tile([1, C], f32)

            # prep-phase PSUM transposes live in their own pool so the banks
            # are free again before the 8 accumulator banks are allocated
            with tc.tile_pool(name="tp_ps", bufs=2, space="PSUM") as tp_ps:
                # ---- X^T tiles (d on partitions) via PE transpose ----
                # all 4 row-chunks of one d-tile land in a single PSUM bank
                # (free offsets 0/128/256/384), then one wide copy casts the
                # whole [128, 512] tile to f16
                for dt in range(NT):
                    tp = tp_ps.tile([P, R], f32, tag="tp")
                    for rt in range(RT):
                        nc.tensor.transpose(
                            tp[:, rt * P:(rt + 1) * P],
                            xr[rt][:, dt * P:(dt + 1) * P], ident[:]
                        )
                    if dt == 3:
                        nc.scalar.copy(xt[dt][:], tp[:])
                    else:
                        nc.vector.tensor_copy(xt[dt][:], tp[:])

                # G1/2 as a row [1, C] for the rank-1 inject: reduce the g
                # tiles directly (only needs the G DMA, so the inject and the
                # whole reduce-MM stream never wait on the gT/ghat chain)
                g1row_ps = tp_ps.tile([1, C], f32, tag="tpg3")
                for t_ in range(NT):
                    nc.tensor.matmul(
                        g1row_ps[:], lhsT=ones_colf[:], rhs=g[t_][:],
                        start=(t_ == 0), stop=(t_ == NT - 1),
                    )
                nc.vector.tensor_scalar_mul(g1row[:], g1row_ps[:], 0.5)

            # ---- row norms 1/|x| (rows layout) ----
            rxn = []
            for k in range(RT):
                sq = scratch.tile([P, D], f32, tag="sq")
                xn2 = small.tile([P, 1], f32, tag="xn2", name=f"xn2_{k}")
                nc.scalar.activation(sq[:], xr[k][:], Act.Square, accum_out=xn2[:])
                xn = small.tile([P, 1], f32, tag="xn", name=f"xn{k}")
                nc.scalar.activation(xn[:], xn2[:], Act.Sqrt)
                r = small.tile([P, 1], f32, tag="rxn", name=f"rxn{k}")
                nc.vector.reciprocal(r[:], xn[:])
                rxn.append(r)

            # -G tiles for the scalar-engine relu bias
            gneg = [gnb_pool.tile([P, C], f32, tag="gneg", name=f"gneg{i}")
                    for i in range(NT)]
            for t_ in range(NT):
                nc.vector.tensor_scalar_mul(gneg[t_][:], g[t_][:], -1.0)

            # ---- accumulators: [n, c] per row-tile ----
            with (
                tc.tile_pool(name="s_ps", bufs=RT, space="PSUM") as s_pool,
                ExitStack() as ctx2,
            ):
                s_ps = [s_pool.tile([P, C], f32, tag="s", name=f"s{k}")
                        for k in range(RT)]
                dot_ps = []

                # S[n, c] = sum_d relu(x - g) + 0.5*G1[c] (broadcast inject)
                for k in range(RT):
                    nc.tensor.matmul(
                        s_ps[k][:],
                        lhsT=ones_row[:],
                        rhs=g1row[:],
                        start=True,
                        stop=False,
                    )
                ghat_state = {}

                def _ghat_step(step):
                    # G-chain pieces dribbled into the instruction stream a
                    # few classes apart: by the time the in-order PE queue
                    # reaches each transpose, its (DVE/Act) inputs are long
                    # done, so the reduce-MM stream never stalls.
                    if step == 0:
                        with tc.tile_pool(name="tpg_ps", bufs=1,
                                          space="PSUM") as tpg_pool:
                            tpg = tpg_pool.tile([C, R], f32, tag="tpg")
                            for tt in range(NT):
                                nc.tensor.transpose(
                                    tpg[:, tt * P:(tt + 1) * P], g[tt][:], ident[:]
                                )
                            nc.vector.tensor_copy(gT[:], tpg[:])
                    elif step == 1:
                        gsq = scratch.tile([C, D], f32, tag="gsq")
                        nc.vector.tensor_tensor(gsq[:], gT[:], gT[:], Alu.mult)
                        gn2 = small.tile([C, 1], f32, tag="gn2")
                        nc.vector.tensor_reduce(gn2[:], gsq[:], Ax.X, Alu.add)
                        gn = small.tile([C, 1], f32, tag="gn")
                        nc.scalar.activation(gn[:], gn2[:], Act.Sqrt)
                        rgn = small.tile([C, 1], f32, tag="rgn")
                        nc.vector.reciprocal(rgn[:], gn[:])
                        ghT = scratch.tile([C, D], f32, tag="ghT")
                        nc.vector.tensor_scalar_mul(ghT[:], gT[:], rgn[:])
                        ghat_state["ghT"] = ghT
                    elif step == 2:
                        ghT = ghat_state["ghT"]
                        with tc.tile_pool(name="tpg2_ps", bufs=1,
                                          space="PSUM") as tpg2_pool:
                            tpg2 = tpg2_pool.tile([P, NT * C], f32, tag="tpg2")
                            for tt in range(NT):
                                nc.tensor.transpose(
                                    tpg2[:, tt * C:(tt + 1) * C],
                                    ghT[:, tt * P:(tt + 1) * P], ident[:C, :C]
                                )
                            nc.vector.tensor_copy(gh_all[:], tpg2[:])
                    elif step == 3:
                        # cosine: DOT[n, c] = sum_d xT[d, n] * ghat[d, c]
                        d_pool = ctx2.enter_context(
                            tc.tile_pool(name="d_ps", bufs=RT, space="PSUM")
                        )
                        for k in range(RT):
                            dot_ps.append(
                                d_pool.tile([P, C], f32, tag="d", name=f"d{k}")
                            )
                        for k in range(RT):
                            for tt in range(NT):
                                nc.tensor.matmul(
                                    dot_ps[k][:],
                                    lhsT=xt[tt][:, k * P:(k + 1) * P],
                                    rhs=gh_all[:, tt * C:(tt + 1) * C],
                                    start=(tt == 0),
                                    stop=(tt == NT - 1),
                                )


                ghat_at = {2: 0, 6: 1, 12: 2, 20: 3}
                for t_ in range(NT):
                    for c in range(C):
                        if t_ == 0 and c in ghat_at:
                            _ghat_step(ghat_at[c])
                        u = u_pool.tile([P, R], f16, tag="u")
                        sel = (t_ * C + c) % 13
                        if sel < 4:
                            nc.scalar.activation(
                                u[:], xt[t_][:], Act.Relu,
                                bias=gneg[t_][:, c:c + 1], scale=1.0,
                            )
                        else:
                            nc.vector.tensor_scalar(
                                u[:], xt[t_][:], g[t_][:, c:c + 1], 0.0,
                                Alu.subtract, Alu.max,
                            )
                        last = (t_ == NT - 1) and (c == C - 1)
                        for k in range(RT):
                            nc.tensor.matmul(
                                s_ps[k][:, c:c + 1],
                                lhsT=u[:, k * P:(k + 1) * P],
                                rhs=ones_col[:],
                                start=False,
                                stop=last,
                            )

                # ---- epilogue per row-tile ----
                for k in range(RT):
                    # cs = dot * (1/|x|)  (1/|g| already folded into ghat)
                    cs = scratch.tile([P, C], f32, tag="cs")
                    nc.vector.tensor_scalar_mul(cs[:], dot_ps[k][:], rxn[k][:])
                    conf = small.tile([P, 1], f32, tag="conf")
                    nc.vector.tensor_reduce(conf[:], cs[:], Ax.X, Alu.max)
                    # confusion = softmax(cs): cs in [-1, 1], no shift needed
                    e2 = scratch.tile([P, C], f32, tag="e2")
                    s2 = small.tile([P, 1], f32, tag="s2")
                    nc.scalar.activation(e2[:], cs[:], Act.Exp, accum_out=s2[:])
                    # norm = softmax(-l1), l1 = 2*S (+ row-constant, dropped)
                    m = small.tile([P, 1], f32, tag="m")
                    nc.vector.tensor_reduce(m[:], s_ps[k][:], Ax.X, Alu.min)
                    m2 = small.tile([P, 1], f32, tag="m2")
                    nc.vector.tensor_scalar_mul(m2[:], m[:], 2.0)
                    e1 = scratch.tile([P, C], f32, tag="e1")
                    s1 = small.tile([P, 1], f32, tag="s1")
                    nc.scalar.activation(
                        e1[:], s_ps[k][:], Act.Exp, bias=m2[:], scale=-2.0,
                        accum_out=s1[:],
                    )
                    # out = conf * (e1/s1) * (e2/s2) = (e1*e2) * (conf/(s1*s2))
                    den = small.tile([P, 1], f32, tag="den")
                    nc.vector.tensor_tensor(den[:], s1[:], s2[:], Alu.mult)
                    rden = small.tile([P, 1], f32, tag="rden")
                    nc.vector.reciprocal(rden[:], den[:])
                    fac = small.tile([P, 1], f32, tag="fac")
                    nc.vector.tensor_tensor(fac[:], conf[:], rden[:], Alu.mult)
                    out_t = out_pool.tile([P, C], f32, tag="out")
                    nc.vector.scalar_tensor_tensor(
                        out_t[:], e1[:], fac[:], e2[:], Alu.mult, Alu.mult
                    )
                    nc.sync.dma_start(Yd[k * P:(k + 1) * P, :], out_t[:])

    _split_excess_waits(nc)
    return nc


def kernel(X: np.ndarray, grp: np.ndarray) -> np.ndarray:
    from concourse.bass_utils import run_bass_kernel_spmd

    if "nc" not in _CACHE:
        _CACHE["nc"] = _build_nc()
    nc = _CACHE["nc"]

    X = np.ascontiguousarray(X, dtype=np.float32)
    g2d = np.ascontiguousarray(grp.reshape(D, C), dtype=np.float32)
    shards = np.split(X, N_CORES, axis=0)
    in_maps = [{"X": s, "G": g2d} for s in shards]
    last_err = None
    for _attempt in range(5):
        try:
            res = run_bass_kernel_spmd(nc, in_maps, list(range(N_CORES)))
            break
        except Exception as e:  # transient device/tunnel hiccups
            last_err = e
            import time
            time.sleep(3.0 + 4.0 * _attempt)
    else:
        raise last_err
    out = np.concatenate(
        [res.results[i]["Y"] for i in range(N_CORES)], axis=0
    )
    return np.ascontiguousarray(out, dtype=np.float32)



# revision 29
# speedup vs baseline: 1.0301x; 1.0301x over previous
"""Trainium2 Bass kernel for the vq_codebook classifier problem.

Computes, for X [4096, 512] f32 and grp [1, 512, 100] f32:
    l1   = sum_d |X[n,d] - grp[0,d,c]|            -> [N, C]
    norm = softmax(-l1, axis=1)
    cs   = (X @ g) / max(|X| * |g|, eps)           (cosine similarity)
    out  = max_c(cs) * softmax(cs, axis=1) * norm

Sharding: data-parallel over N across 8 NeuronCores (512 rows each),
grp replicated.

Math notes used by the kernel:
  |v| = 2*relu(v) - v, so
  l1[n,c] = 2*sum_d relu(x-g) - sum_d x + sum_d g
The "sum_d x" term is constant over classes and cancels inside
softmax(-l1), so it is dropped. sum_d g (G1) is injected into the same
PSUM accumulator via a rank-1 matmul (ones-row x 0.5*G1).

relu(x - g) tiles are produced in a d-on-partition layout by the vector
engine (tensor_scalar sub+max, f16 2x mode, ~69% of tiles) and the
scalar engine (activation Relu with per-partition bias -g, ~31%). The
partition (d) reduction runs on TensorE with the relu tile as the
*stationary* operand (f16 fast-weight-load) and a ones-column as the
moving operand, so each class lands in one PSUM *column* of an [n, c]
accumulator (PE outputs may only start at partition 0/32/64, but
free-dim offsets are unrestricted; this also leaves results
pre-transposed for the per-row epilogue).

Scheduling notes (the engines are all near-saturated, so ordering is
what matters):
  - reps>1 (timing harness) runs the whole body in a tc.For_i hardware
    loop: NEFF size, and thus host-side per-call cost, is independent
    of reps, so the wall-clock differential isolates device time.
  - ~32 f16 warm-up matmuls at the head of each rep trip the PE HAM
    clock gate (1.2 -> 2.4 GHz) while the input DMAs land.
  - The X^T transposes write four row-chunks into one PSUM bank so a
    single wide copy casts each [128, 512] tile to f16.
  - The cosine/ghat chain (G^T, column norms, ghat, dot matmuls) is
    dribbled into the class loop a few classes apart; the in-order PE
    queue reaches each piece long after its inputs are ready, so the
    1600-matmul reduce stream never stalls on it.
  - The G1 inject row is reduced straight from the g tiles with four
    ones-stationary matmuls, so it only depends on the G DMA.
  - Deep u-tile pool (48) lets producers run ~12 us ahead of the PE.
"""

import numpy as np

P = 128
R = 512          # rows per core (4096 / 8 cores)
D = 512
C = 100
NT = D // P      # 4 d-tiles
RT = R // P      # 4 row-tiles
N_CORES = 8

_CACHE = {}
import os as _os
WARMUP_MMS = int(_os.environ.get("WARMUP_MMS", "32"))


def _split_excess_waits(nc, limit=1):
    """walrus in this container rejects instructions carrying more than
    one sync wait ("Too many sync wait commands"). Hoist excess waits
    onto same-engine NoOps inserted immediately before the instruction."""
    import concourse.mybir as mb
    import bass_rust

    n_id = [0]

    def mknop(engine, waits):
        n_id[0] += 1
        return bass_rust.InstNoOp(
            name=f"waitsplit-{n_id[0]}", engine=engine, ins=[], outs=[],
            sync_info=mb.SyncInfo(on_wait=list(waits), on_update=[]),
        )

    for fn in nc.m.functions:
        for bb in fn.blocks:
            insts = bb.instructions
            out = []
            for inst in insts:
                si = inst.sync_info
                if si is not None and si.on_wait and len(si.on_wait) > limit:
                    waits = list(si.on_wait)
                    extra, keep = waits[:-limit], waits[-limit:]
                    for w in extra:
                        out.append(mknop(inst.engine, [w]))
                    inst.sync_info = mb.SyncInfo(
                        on_wait=keep, on_update=list(si.on_update)
                    )
                out.append(inst)
            insts[:] = out


def _build_nc(reps: int = 1):
    import concourse.bass as bass
    import concourse.mybir as mybir
    import concourse.tile as tile
    from concourse.masks import make_identity
    from contextlib import ExitStack

    f32 = mybir.dt.float32
    f16 = mybir.dt.float16
    Alu = mybir.AluOpType
    Act = mybir.ActivationFunctionType
    Ax = mybir.AxisListType

    nc = bass.Bass(target_bir_lowering=False)
    Xd = nc.declare_dram_parameter("X", [R, D], f32, isOutput=False)
    Gd = nc.declare_dram_parameter("G", [D, C], f32, isOutput=False)
    Yd = nc.declare_dram_parameter("Y", [R, C], f32, isOutput=True)

    with ExitStack() as ctx:
        tc = ctx.enter_context(tile.TileContext(nc))
        consts = ctx.enter_context(tc.tile_pool(name="consts", bufs=1))
        xr_pool = ctx.enter_context(tc.tile_pool(name="xr", bufs=RT))
        xt_pool = ctx.enter_context(tc.tile_pool(name="xt", bufs=NT))
        g_pool = ctx.enter_context(tc.tile_pool(name="g", bufs=NT))
        gnb_pool = ctx.enter_context(tc.tile_pool(name="gnb", bufs=NT))
        gh_pool = ctx.enter_context(tc.tile_pool(name="gh", bufs=NT))
        small = ctx.enter_context(tc.tile_pool(name="small", bufs=24))
        scratch = ctx.enter_context(tc.tile_pool(name="scratch", bufs=2))
        u_pool = ctx.enter_context(tc.tile_pool(name="u", bufs=48))
        out_pool = ctx.enter_context(tc.tile_pool(name="out", bufs=RT))

        from contextlib import nullcontext

        def _rep_scope():
            # reps>1 exists only for the timing harness: run the whole
            # computation `reps` times on-device via a hardware loop so the
            # NEFF (and thus host-side lowering/transfer cost) stays
            # constant in `reps` and the differential isolates HW time.
            return tc.For_i(0, reps, 1) if reps > 1 else nullcontext()

        # ---- constants (loop-invariant; built once) ----
        ident = consts.tile([P, P], f32)
        make_identity(nc, ident[:])
        ones_col = consts.tile([P, 1], f16)
        nc.vector.memset(ones_col[:], 1.0)
        ones_row = consts.tile([1, P], f32)
        nc.vector.memset(ones_row[:], 1.0)
        ones_colf = consts.tile([P, 1], f32)
        nc.vector.memset(ones_colf[:], 1.0)
        ident16 = consts.tile([P, P], f16)
        nc.vector.tensor_copy(ident16[:], ident[:])

        with _rep_scope():
            # ---- load inputs (spread across the two hwdge rings) ----
            xr = []
            for k in range(RT):
                t = xr_pool.tile([P, D], f32, tag="xr", name=f"xr{k}")
                eng = nc.sync if k % 2 == 0 else nc.scalar
                eng.dma_start(t[:], Xd[k * P:(k + 1) * P, :])
                xr.append(t)
            g = []
            for t_ in range(NT):
                gt = g_pool.tile([P, C], f32, tag="g", name=f"g{t_}")
                nc.sync.dma_start(gt[:], Gd[t_ * P:(t_ + 1) * P, :])
                g.append(gt)

            # warm the PE clock gate (HAM) while the DMAs land: ~40 dummy
            # matmuls on a constant tile keep the PE busy through the first
            # activity window so the real matmuls run at 2.4 GHz
            with tc.tile_pool(name="warm_ps", bufs=1, space="PSUM") as warm_ps:
                wps = warm_ps.tile([P, P], f32, tag="warm")
                for _w in range(WARMUP_MMS):
                    nc.tensor.matmul(
                        wps[:], lhsT=ident16[:], rhs=ident16[:],
                        start=True, stop=True,
                    )

            xt = [xt_pool.tile([P, R], f16, tag="xt", name=f"xt{i}") for i in range(NT)]
            gT = consts.tile([C, D], f32)
            gh_all = gh_pool.tile([P, NT * C], f16, tag="gh", name="gh_all")
            g1row = consts.tile([1, C], f32)

            # prep-phase PSUM transposes live in their own pool so the banks
            # are free again before the 8 accumulator banks are allocated
            with tc.tile_pool(name="tp_ps", bufs=2, space="PSUM") as tp_ps:
                # ---- X^T tiles (d on partitions) via PE transpose ----
                # all 4 row-chunks of one d-tile land in a single PSUM bank
                # (free offsets 0/128/256/384), then one wide copy casts the
                # whole [128, 512] tile to f16
                for dt in range(NT):
                    tp = tp_ps.tile([P, R], f32, tag="tp")
                    for rt in range(RT):
                        nc.tensor.transpose(
                            tp[:, rt * P:(rt + 1) * P],
                            xr[rt][:, dt * P:(dt + 1) * P], ident[:]
                        )
                    if dt == 3:
                        nc.scalar.copy(xt[dt][:], tp[:])
                    else:
                        nc.vector.tensor_copy(xt[dt][:], tp[:])

                # G1/2 as a row [1, C] for the rank-1 inject: reduce the g
                # tiles directly (only needs the G DMA, so the inject and the
                # whole reduce-MM stream never wait on the gT/ghat chain)
                g1row_ps = tp_ps.tile([1, C], f32, tag="tpg3")
                for t_ in range(NT):
                    nc.tensor.matmul(
                        g1row_ps[:], lhsT=ones_colf[:], rhs=g[t_][:],
                        start=(t_ == 0), stop=(t_ == NT - 1),
                    )
                nc.vector.tensor_scalar_mul(g1row[:], g1row_ps[:], 0.5)

            # ---- row norms 1/|x| (rows layout) ----
            rxn = []
            for k in range(RT):
                sq = scratch.tile([P, D], f32, tag="sq")
                xn2 = small.tile([P, 1], f32, tag="xn2", name=f"xn2_{k}")
                nc.scalar.activation(sq[:], xr[k][:], Act.Square, accum_out=xn2[:])
                xn = small.tile([P, 1], f32, tag="xn", name=f"xn{k}")
                nc.scalar.activation(xn[:], xn2[:], Act.Sqrt)
                r = small.tile([P, 1], f32, tag="rxn", name=f"rxn{k}")
                nc.vector.reciprocal(r[:], xn[:])
                rxn.append(r)

            # -G tiles for the scalar-engine relu bias
            gneg = [gnb_pool.tile([P, C], f32, tag="gneg", name=f"gneg{i}")
                    for i in range(NT)]
            for t_ in range(NT):
                nc.vector.tensor_scalar_mul(gneg[t_][:], g[t_][:], -1.0)

            # ---- accumulators: [n, c] per row-tile ----
            with (
                tc.tile_pool(name="s_ps", bufs=RT, space="PSUM") as s_pool,
                ExitStack() as ctx2,
            ):
                s_ps = [s_pool.tile([P, C], f32, tag="s", name=f"s{k}")
                        for k in range(RT)]
                dot_ps = []

                # S[n, c] = sum_d relu(x - g) + 0.5*G1[c] (broadcast inject)
                for k in range(RT):
                    nc.tensor.matmul(
                        s_ps[k][:],
                        lhsT=ones_row[:],
                        rhs=g1row[:],
                        start=True,
                        stop=False,
                    )
                ghat_state = {}

                def _ghat_step(step):
                    # G-chain pieces dribbled into the instruction stream a
                    # few classes apart: by the time the in-order PE queue
                    # reaches each transpose, its (DVE/Act) inputs are long
                    # done, so the reduce-MM stream never stalls.
                    if step == 0:
                        with tc.tile_pool(name="tpg_ps", bufs=1,
                                          space="PSUM") as tpg_pool:
                            tpg = tpg_pool.tile([C, R], f32, tag="tpg")
                            for tt in range(NT):
                                nc.tensor.transpose(
                                    tpg[:, tt * P:(tt + 1) * P], g[tt][:], ident[:]
                                )
                            nc.vector.tensor_copy(gT[:], tpg[:])
                    elif step == 1:
                        gsq = scratch.tile([C, D], f32, tag="gsq")
                        nc.vector.tensor_tensor(gsq[:], gT[:], gT[:], Alu.mult)
                        gn2 = small.tile([C, 1], f32, tag="gn2")
                        nc.vector.tensor_reduce(gn2[:], gsq[:], Ax.X, Alu.add)
                        gn = small.tile([C, 1], f32, tag="gn")
                        nc.scalar.activation(gn[:], gn2[:], Act.Sqrt)
                        rgn = small.tile([C, 1], f32, tag="rgn")
                        nc.vector.reciprocal(rgn[:], gn[:])
                        ghT = scratch.tile([C, D], f32, tag="ghT")
                        nc.vector.tensor_scalar_mul(ghT[:], gT[:], rgn[:])
                        ghat_state["ghT"] = ghT
                    elif step == 2:
                        ghT = ghat_state["ghT"]
                        with tc.tile_pool(name="tpg2_ps", bufs=1,
                                          space="PSUM") as tpg2_pool:
                            tpg2 = tpg2_pool.tile([P, NT * C], f32, tag="tpg2")
                            for tt in range(NT):
                                nc.tensor.transpose(
                                    tpg2[:, tt * C:(tt + 1) * C],
                                    ghT[:, tt * P:(tt + 1) * P], ident[:C, :C]
                                )
                            nc.vector.tensor_copy(gh_all[:], tpg2[:])
                    elif step == 3:
                        # cosine: DOT[n, c] = sum_d xT[d, n] * ghat[d, c]
                        d_pool = ctx2.enter_context(
                            tc.tile_pool(name="d_ps", bufs=RT, space="PSUM")
                        )
                        for k in range(RT):
                            dot_ps.append(
                                d_pool.tile([P, C], f32, tag="d", name=f"d{k}")
                            )
                        for k in range(RT):
                            for tt in range(NT):
                                nc.tensor.matmul(
                                    dot_ps[k][:],
                                    lhsT=xt[tt][:, k * P:(k + 1) * P],
                                    rhs=gh_all[:, tt * C:(tt + 1) * C],
                                    start=(tt == 0),
                                    stop=(tt == NT - 1),
                                )


                ghat_at = {2: 0, 6: 1, 12: 2, 20: 3}
                for t_ in range(NT):
                    for c in range(C):
                        if t_ == 0 and c in ghat_at:
                            _ghat_step(ghat_at[c])
                        u = u_pool.tile([P, R], f16, tag="u")
                        sel = (t_ * C + c) % 13
                        if sel < 4:
                            nc.scalar.activation(
                                u[:], xt[t_][:], Act.Relu,
                                bias=gneg[t_][:, c:c + 1], scale=1.0,
                            )
                        else:
                            nc.vector.tensor_scalar(
                                u[:], xt[t_][:], g[t_][:, c:c + 1], 0.0,
                                Alu.subtract, Alu.max,
                            )
                        last = (t_ == NT - 1) and (c == C - 1)
                        for k in range(RT):
                            nc.tensor.matmul(
                                s_ps[k][:, c:c + 1],
                                lhsT=u[:, k * P:(k + 1) * P],
                                rhs=ones_col[:],
                                start=False,
                                stop=last,
                            )

                # ---- epilogue per row-tile ----
                for k in range(RT):
                    # cs = dot * (1/|x|)  (1/|g| already folded into ghat)
                    cs = scratch.tile([P, C], f32, tag="cs")
                    nc.vector.tensor_scalar_mul(cs[:], dot_ps[k][:], rxn[k][:])
                    conf = small.tile([P, 1], f32, tag="conf")
                    nc.vector.tensor_reduce(conf[:], cs[:], Ax.X, Alu.max)
                    # confusion = softmax(cs): cs in [-1, 1], no shift needed
                    e2 = scratch.tile([P, C], f32, tag="e2")
                    s2 = small.tile([P, 1], f32, tag="s2")
                    nc.scalar.activation(e2[:], cs[:], Act.Exp, accum_out=s2[:])
                    # norm = softmax(-l1), l1 = 2*S (+ row-constant, dropped)
                    m = small.tile([P, 1], f32, tag="m")
                    nc.vector.tensor_reduce(m[:], s_ps[k][:], Ax.X, Alu.min)
                    m2 = small.tile([P, 1], f32, tag="m2")
                    nc.vector.tensor_scalar_mul(m2[:], m[:], 2.0)
                    e1 = scratch.tile([P, C], f32, tag="e1")
                    s1 = small.tile([P, 1], f32, tag="s1")
                    nc.scalar.activation(
                        e1[:], s_ps[k][:], Act.Exp, bias=m2[:], scale=-2.0,
                        accum_out=s1[:],
                    )
                    # out = conf * (e1/s1) * (e2/s2) = (e1*e2) * (conf/(s1*s2))
                    den = small.tile([P, 1], f32, tag="den")
                    nc.vector.tensor_tensor(den[:], s1[:], s2[:], Alu.mult)
                    rden = small.tile([P, 1], f32, tag="rden")
                    nc.vector.reciprocal(rden[:], den[:])
                    fac = small.tile([P, 1], f32, tag="fac")
                    nc.vector.tensor_tensor(fac[:], conf[:], rden[:], Alu.mult)
                    out_t = out_pool.tile([P, C], f32, tag="out")
                    nc.vector.scalar_tensor_tensor(
                        out_t[:], e1[:], fac[:], e2[:], Alu.mult, Alu.mult
                    )
                    nc.sync.dma_start(Yd[k * P:(k + 1) * P, :], out_t[:])

    _split_excess_waits(nc)
    return nc


def kernel(X: np.ndarray, grp: np.ndarray) -> np.ndarray:
    from concourse.bass_utils import run_bass_kernel_spmd

    if "nc" not in _CACHE:
        _CACHE["nc"] = _build_nc()
    nc = _CACHE["nc"]

    X = np.ascontiguousarray(X, dtype=np.float32)
    g2d = np.ascontiguousarray(grp.reshape(D, C), dtype=np.float32)
    shards = np.split(X, N_CORES, axis=0)
    in_maps = [{"X": s, "G": g2d} for s in shards]
    last_err = None
    for _attempt in range(5):
        try:
            res = run_bass_kernel_spmd(nc, in_maps, list(range(N_CORES)))
            break
        except Exception as e:  # transient device/tunnel hiccups
            last_err = e
            import time
            time.sleep(3.0 + 4.0 * _attempt)
    else:
        raise last_err
    out = np.concatenate(
        [res.results[i]["Y"] for i in range(N_CORES)], axis=0
    )
    return np.ascontiguousarray(out, dtype=np.float32)

